# revision 7
# baseline (speedup 1.0000x reference)
"""Trainium2 Bass kernel for nn_AggregPolicy (GNN message passing / GRU chain).

Strategy:
  - Pure data parallelism: 524288 rows split across 8 cores (65536 each).
  - Feature-major on-chip layout: state s = [hj0..hj6, hm] (32 features) on
    partitions, batch on the free dim. 4 batch subgroups stacked on partitions
    (partition 32q+s) so elementwise ops use all 128 lanes.
  - Each GRU message-passing step's full linear algebra is a 32->128 linear map
    (neighbor structure folded into a banded weight matrix). Executed as 16
    small matmuls (K=32, M=32) with tile_position packing, writing gate-type-
    major PSUM banks: R | Z | INN | HN, each [128, 512].
  - Nonlinearities: ACT sigmoid/tanh with fused per-partition bias; DVE/GPSIMD
    for the remaining pointwise ops.
  - Iteration 1 consumes x directly (initial Linear layers folded into the
    first step's weights); final Linear folded into 4 output matmuls; final
    bias + layout restore on host.
"""

import sys
import numpy as np

for _p in ("/opt/trn_rl_repo",):
    if _p not in sys.path:
        sys.path.append(_p)

import ml_dtypes
from contextlib import ExitStack

import concourse.bass as bass
import concourse.bacc as bacc
import concourse.tile as tile
from concourse import mybir
from concourse.bass_utils import run_bass_kernel_spmd

BF16 = ml_dtypes.bfloat16
AF = mybir.ActivationFunctionType
ALU = mybir.AluOpType

N_CORES = 8
B = 524288
BC = B // N_CORES          # rows per core = 65536
NSUB = 4                   # batch subgroups stacked on partitions
NCOL = BC // NSUB          # free-dim columns per subgroup = 16384
CT = 512                   # columns per supertile (one PSUM bank)
NT = NCOL // CT            # 32 supertiles
H = 4
NU = 8                     # 7 joints + master
S = 32                     # state features


def _gate_blocks(p):
    """Build the 32->128 banded linear map for one message-passing step.

    Returns W (gate-major blocks) [4][32, 32] mapping state->gates and the
    four per-partition bias vectors (within one 32-wide subgroup block).
    Gate blocks: 0=R(sum), 1=Z(sum), 2=INN (input side of n), 3=HN (hidden
    side of n, bias excluded -- applied via STT scalar).
    State layout: [hj0(4) .. hj6(4), hm(4)].
    """
    Wih_j, Whh_j = p["Wih_j"], p["Whh_j"]
    Wih_m, Whh_m = p["Wih_m"], p["Whh_m"]
    W = [np.zeros((S, S), np.float64) for _ in range(4)]

    def st(u):  # state slice of unit u
        return slice(4 * u, 4 * u + 4)

    for u in range(7):
        left = None if u == 0 else st(u - 1)   # u==0 -> hm
        right = None if u == 6 else st(u + 1)  # u==6 -> zero
        for g, rows in ((0, slice(0, 4)), (1, slice(4, 8))):
            # sum gates: Wih(left,right) + Whh(self)
            Wl = Wih_j[rows, 0:4]
            Wr = Wih_j[rows, 4:8]
            Wh = Whh_j[rows, :]
            tgt = W[g][st(u), :]
            if left is None:
                tgt[:, 28:32] += Wl
            else:
                tgt[:, left] += Wl
            if right is not None:
                tgt[:, right] += Wr
            tgt[:, st(u)] += Wh
        # INN: input side only
        rows = slice(8, 12)
        tgt = W[2][st(u), :]
        if u == 0:
            tgt[:, 28:32] += Wih_j[rows, 0:4]
        else:
            tgt[:, st(u - 1)] += Wih_j[rows, 0:4]
        if u != 6:
            tgt[:, st(u + 1)] += Wih_j[rows, 4:8]
        # HN: hidden side only
        W[3][st(u), st(u)] += Whh_j[rows, :]

    # master unit (index 7, state rows 28:32); input = hj0, hidden = hm
    for g, rows in ((0, slice(0, 4)), (1, slice(4, 8))):
        W[g][28:32, 0:4] += Wih_m[rows, :]
        W[g][28:32, 28:32] += Whh_m[rows, :]
    W[2][28:32, 0:4] += Wih_m[8:12, :]
    W[3][28:32, 28:32] += Whh_m[8:12, :]

    def unit_bias(vec_j, vec_m, rows):
        b = np.zeros(S, np.float64)
        for u in range(7):
            b[st(u)] = vec_j[rows]
        b[28:32] = vec_m[rows]
        return b

    br = unit_bias(p["bih_j"], p["bih_m"], slice(0, 4)) + unit_bias(
        p["bhh_j"], p["bhh_m"], slice(0, 4))
    bz = unit_bias(p["bih_j"], p["bih_m"], slice(4, 8)) + unit_bias(
        p["bhh_j"], p["bhh_m"], slice(4, 8))
    binn = unit_bias(p["bih_j"], p["bih_m"], slice(8, 12))
    bhn = unit_bias(p["bhh_j"], p["bhh_m"], slice(8, 12))
    return W, (br, bz, binn, bhn)


def _a0_ext(p):
    """[32, 19] initial-linear map: state0 = A0e @ [x(18); 1]."""
    A = np.zeros((S, 19), np.float64)
    Wj, bj, Wm, bm = p["Wj"], p["bj"], p["Wm"], p["bm"]
    for u in range(7):
        A[4 * u:4 * u + 4, 4 + u] = Wj[:, 0]
        A[4 * u:4 * u + 4, 11 + u] = Wj[:, 1]
        A[4 * u:4 * u + 4, 18] = bj
    A[28:32, 0:4] = Wm
    A[28:32, 18] = bm
    return A


def _host_weights(inputs):
    p = {k: np.asarray(v, np.float64) for k, v in inputs.items() if k != "x"}
    W, (br, bz, binn, bhn) = _gate_blocks(p)
    A0e = _a0_ext(p)

    # wtb [128,128]: rows 32q+k (k<32) = state idx, cols 32g+m = gate out m of block g
    wtb = np.zeros((128, 128), np.float64)
    # wt1 [128,128]: iteration-1 gate weights consuming xe(19) directly
    wt1 = np.zeros((128, 128), np.float64)
    # a0t: diag blocks for S0 psum (iter-1 blend h operand)
    a0t = np.zeros((128, 128), np.float64)
    # wat: diag blocks for output linear (state -> 7 activations)
    wat = np.zeros((128, 128), np.float64)
    Wa = p["Wa"]  # [1, 4]
    for q in range(4):
        r0 = 32 * q
        for g in range(4):
            wtb[r0:r0 + 32, 32 * g:32 * g + 32] = W[g].T
            W1g = W[g] @ A0e  # [32, 19]
            wt1[r0:r0 + 19, 32 * g:32 * g + 32] = W1g.T
        a0t[r0:r0 + 19, r0:r0 + 32] = A0e.T
        for u in range(7):
            wat[r0 + 4 * u:r0 + 4 * u + 4, r0 + u] = Wa[0, :]

    def bias128(v):
        return np.tile(v, 4).astype(np.float32).reshape(128, 1)

    return {
        "wtb": wtb.astype(BF16), "wt1": wt1.astype(BF16),
        "a0t": a0t.astype(BF16), "wat": wat.astype(BF16),
        "br": bias128(br), "bz": bias128(bz),
        "binn": bias128(binn), "bhn": bias128(bhn),
    }, float(np.asarray(inputs["ba"]).reshape(-1)[0])


def _host_x(x):
    """x [B,18] fp32 -> per-core [128, NCOL] bf16 (partition 32q+k, k<19)."""
    xs = []
    for c in range(N_CORES):
        xc = np.asarray(x[c * BC:(c + 1) * BC], np.float32)
        arr = np.zeros((4, 32, NCOL), np.float32)
        arr[:, 0:18, :] = xc.reshape(4, NCOL, 18).transpose(0, 2, 1)
        arr[:, 18, :] = 1.0
        xs.append(arr.reshape(128, NCOL).astype(BF16))
    return xs


def _build_program(ncol=NCOL, nt=NT):
    nc = bacc.Bacc("TRN2", target_bir_lowering=False, debug=False,
                   num_devices=N_CORES)
    f32 = mybir.dt.float32
    bf16 = mybir.dt.bfloat16

    xd = nc.dram_tensor("x_il", [128, ncol], bf16, kind="ExternalInput").ap()
    wtbd = nc.dram_tensor("wtb", [128, 128], bf16, kind="ExternalInput").ap()
    wt1d = nc.dram_tensor("wt1", [128, 128], bf16, kind="ExternalInput").ap()
    a0td = nc.dram_tensor("a0t", [128, 128], bf16, kind="ExternalInput").ap()
    watd = nc.dram_tensor("wat", [128, 128], bf16, kind="ExternalInput").ap()
    biasd = {k: nc.dram_tensor(k, [128, 1], f32, kind="ExternalInput").ap()
             for k in ("br", "bz", "binn", "bhn")}
    yd = nc.dram_tensor("y", [28, ncol], f32, kind="ExternalOutput").ap()

    with tile.TileContext(nc) as tc, ExitStack() as ctx:
        cpool = ctx.enter_context(tc.tile_pool(name="consts", bufs=1))
        spool = ctx.enter_context(tc.tile_pool(name="state", bufs=1))
        gpool = ctx.enter_context(tc.tile_pool(name="gates", bufs=3))
        opool = ctx.enter_context(tc.tile_pool(name="outsb", bufs=3))

        xt = spool.tile([128, ncol], bf16, tag="xt")
        nc.sync.dma_start(xt[:], xd[:])
        st = spool.tile([128, ncol], bf16, tag="st")

        wtb = cpool.tile([128, 128], bf16, tag="wtb")
        nc.sync.dma_start(wtb[:], wtbd[:])
        wt1 = cpool.tile([128, 128], bf16, tag="wt1")
        nc.sync.dma_start(wt1[:], wt1d[:])
        a0t = cpool.tile([128, 128], bf16, tag="a0t")
        nc.sync.dma_start(a0t[:], a0td[:])
        wat = cpool.tile([128, 128], bf16, tag="wat")
        nc.sync.dma_start(wat[:], watd[:])
        bias = {}
        for k in ("br", "bz", "binn", "bhn"):
            bias[k] = cpool.tile([128, 1], f32, tag=k, name=f"b_{k}")
            nc.sync.dma_start(bias[k][:], biasd[k][:])

        def step(it, psg, psum_s0_pool):
            """One message-passing iteration over all supertiles."""
            first = it == 0
            wt = wt1 if first else wtb
            kk = 19 if first else 32
            for t in range(nt):
                cols = slice(t * CT, (t + 1) * CT)
                G = psg.tile([128, 4 * CT], f32, tag="G")
                rhs_t = xt if first else st
                for g in range(4):
                    for q in range(4):
                        r0 = 32 * q
                        nc.tensor.matmul(
                            G[r0:r0 + 32, g * CT:(g + 1) * CT],
                            wt[r0:r0 + kk, 32 * g:32 * g + 32],
                            rhs_t[r0:r0 + kk, cols],
                            start=True, stop=True,
                            tile_position=(r0, r0),
                        )
                if first:
                    S0 = psum_s0_pool.tile([128, CT], f32, tag="S0")
                    for q in range(4):
                        r0 = 32 * q
                        nc.tensor.matmul(
                            S0[r0:r0 + 32, :],
                            a0t[r0:r0 + 19, r0:r0 + 32],
                            xt[r0:r0 + 19, cols],
                            start=True, stop=True,
                            tile_position=(r0, r0),
                        )
                r = gpool.tile([128, CT], bf16, tag="r")
                nc.scalar.activation(r[:], G[:, 0:CT], AF.Sigmoid,
                                     bias=bias["br"][:])
                z = gpool.tile([128, CT], bf16, tag="z")
                nc.scalar.activation(z[:], G[:, CT:2 * CT], AF.Sigmoid,
                                     bias=bias["bz"][:])
                t1 = gpool.tile([128, CT], bf16, tag="t1")
                nc.vector.scalar_tensor_tensor(
                    t1[:], G[:, 3 * CT:4 * CT], bias["bhn"][:], r[:],
                    ALU.add, ALU.mult)
                t2 = gpool.tile([128, CT], bf16, tag="t2")
                nc.vector.tensor_add(t2[:], t1[:], G[:, 2 * CT:3 * CT])
                n = gpool.tile([128, CT], bf16, tag="n")
                nc.scalar.activation(n[:], t2[:], AF.Tanh,
                                     bias=bias["binn"][:])
                d = gpool.tile([128, CT], bf16, tag="d")
                if first:
                    nc.vector.tensor_sub(d[:], S0[:], n[:])
                else:
                    nc.gpsimd.tensor_sub(d[:], st[:, cols], n[:])
                e = gpool.tile([128, CT], bf16, tag="e")
                nc.gpsimd.tensor_mul(e[:], z[:], d[:])
                nc.vector.tensor_add(st[:, cols], n[:], e[:])

        with tc.tile_pool(name="ps1", bufs=1, space="PSUM") as psg1, \
             tc.tile_pool(name="ps0", bufs=2, space="PSUM") as ps0:
            step(0, psg1, ps0)
        with tc.tile_pool(name="psg", bufs=2, space="PSUM") as psg:
            for it in range(1, 7):
                step(it, psg, None)
        with tc.tile_pool(name="pso", bufs=2, space="PSUM") as pso:
            for t in range(nt):
                cols = slice(t * CT, (t + 1) * CT)
                O = pso.tile([128, CT], f32, tag="O")
                for q in range(4):
                    r0 = 32 * q
                    nc.tensor.matmul(
                        O[r0:r0 + 32, :],
                        wat[r0:r0 + 28, r0:r0 + 32],
                        st[r0:r0 + 28, cols],
                        start=True, stop=True,
                        tile_position=(r0, r0),
                    )
                osb = opool.tile([128, CT], f32, tag="osb")
                nc.scalar.copy(osb[:], O[:])
                for q in range(4):
                    nc.sync.dma_start(yd[7 * q:7 * q + 7, cols],
                                      osb[32 * q:32 * q + 7, :])

    nc.compile()
    return nc


_NC_CACHE = {}


def kernel(**inputs):
    x = np.asarray(inputs["x"])
    wd, ba = _host_weights(inputs)
    xs = _host_x(x)

    if "prog" not in _NC_CACHE:
        _NC_CACHE["prog"] = _build_program()
    nc = _NC_CACHE["prog"]

    in_maps = []
    for c in range(N_CORES):
        m = {"x_il": xs[c]}
        m.update({k: wd[k] for k in ("wtb", "wt1", "a0t", "wat",
                                     "br", "bz", "binn", "bhn")})
        in_maps.append(m)

    res = run_bass_kernel_spmd(nc, in_maps, core_ids=list(range(N_CORES)))
    _NC_CACHE["last_result"] = res
    outs = []
    for c in range(N_CORES):
        yc = np.asarray(res.results[c]["y"], np.float32)  # [28, NCOL]
        oc = yc.reshape(4, 7, NCOL).transpose(0, 2, 1).reshape(BC, 7)
        outs.append(oc)
    out = np.concatenate(outs, 0).reshape(B, 7, 1) + np.float32(ba)
    return out.astype(np.float32)


if __name__ == "__main__":
    rng = np.random.default_rng(0)
    demo = {"x": rng.standard_normal((B, 18), dtype=np.float32)}
    for k, shp in [("Wj", (H, 2)), ("bj", (H,)), ("Wm", (H, H)), ("bm", (H,)),
                   ("Wih_j", (3 * H, 2 * H)), ("Whh_j", (3 * H, H)),
                   ("bih_j", (3 * H,)), ("bhh_j", (3 * H,)),
                   ("Wih_m", (3 * H, H)), ("Whh_m", (3 * H, H)),
                   ("bih_m", (3 * H,)), ("bhh_m", (3 * H,)),
                   ("Wa", (1, H)), ("ba", (1,))]:
        demo[k] = (rng.standard_normal(shp) * 0.1).astype(np.float32)
    y = kernel(**demo)
    print(y.shape, y.dtype)


# revision 12
# speedup vs baseline: 1.0059x; 1.0059x over previous
"""Trainium2 Bass kernel for nn_AggregPolicy (GNN message passing / GRU chain).

Strategy:
  - Pure data parallelism: 524288 rows split across 8 cores (65536 each).
  - Feature-major on-chip layout: state s = [hj0..hj6, hm] (32 features) on
    partitions, batch on the free dim. 4 batch subgroups stacked on partitions
    (partition 32q+s) so elementwise ops use all 128 lanes.
  - Each GRU message-passing step's full linear algebra is a 32->128 linear map
    (neighbor structure folded into a banded weight matrix). Executed as 16
    small matmuls (K=32, M=32) with tile_position packing, writing gate-type-
    major PSUM banks: R | Z | INN | HN, each [128, 512].
  - Nonlinearities: ACT sigmoid/tanh with fused per-partition bias; DVE/GPSIMD
    for the remaining pointwise ops.
  - Iteration 1 consumes x directly (initial Linear layers folded into the
    first step's weights); final Linear folded into 4 output matmuls; final
    bias + layout restore on host.
"""

import sys
import numpy as np

for _p in ("/opt/trn_rl_repo",):
    if _p not in sys.path:
        sys.path.append(_p)

import ml_dtypes
from contextlib import ExitStack

import concourse.bass as bass
import concourse.bacc as bacc
import concourse.tile as tile
from concourse import mybir
from concourse.bass_utils import run_bass_kernel_spmd

BF16 = ml_dtypes.bfloat16
AF = mybir.ActivationFunctionType
ALU = mybir.AluOpType

N_CORES = 8
B = 524288
BC = B // N_CORES          # rows per core = 65536
NSUB = 4                   # batch subgroups stacked on partitions
NCOL = BC // NSUB          # free-dim columns per subgroup = 16384
CT = 512                   # columns per supertile (one PSUM bank)
NT = NCOL // CT            # 32 supertiles
H = 4
NU = 8                     # 7 joints + master
S = 32                     # state features


def _gate_blocks(p):
    """Build the 32->128 banded linear map for one message-passing step.

    Returns W (gate-major blocks) [4][32, 32] mapping state->gates and the
    four per-partition bias vectors (within one 32-wide subgroup block).
    Gate blocks: 0=R(sum), 1=Z(sum), 2=INN (input side of n), 3=HN (hidden
    side of n, bias excluded -- applied via STT scalar).
    State layout: [hj0(4) .. hj6(4), hm(4)].
    """
    Wih_j, Whh_j = p["Wih_j"], p["Whh_j"]
    Wih_m, Whh_m = p["Wih_m"], p["Whh_m"]
    W = [np.zeros((S, S), np.float64) for _ in range(4)]

    def st(u):  # state slice of unit u
        return slice(4 * u, 4 * u + 4)

    for u in range(7):
        left = None if u == 0 else st(u - 1)   # u==0 -> hm
        right = None if u == 6 else st(u + 1)  # u==6 -> zero
        for g, rows in ((0, slice(0, 4)), (1, slice(4, 8))):
            # sum gates: Wih(left,right) + Whh(self)
            Wl = Wih_j[rows, 0:4]
            Wr = Wih_j[rows, 4:8]
            Wh = Whh_j[rows, :]
            tgt = W[g][st(u), :]
            if left is None:
                tgt[:, 28:32] += Wl
            else:
                tgt[:, left] += Wl
            if right is not None:
                tgt[:, right] += Wr
            tgt[:, st(u)] += Wh
        # INN: input side only
        rows = slice(8, 12)
        tgt = W[2][st(u), :]
        if u == 0:
            tgt[:, 28:32] += Wih_j[rows, 0:4]
        else:
            tgt[:, st(u - 1)] += Wih_j[rows, 0:4]
        if u != 6:
            tgt[:, st(u + 1)] += Wih_j[rows, 4:8]
        # HN: hidden side only
        W[3][st(u), st(u)] += Whh_j[rows, :]

    # master unit (index 7, state rows 28:32); input = hj0, hidden = hm
    for g, rows in ((0, slice(0, 4)), (1, slice(4, 8))):
        W[g][28:32, 0:4] += Wih_m[rows, :]
        W[g][28:32, 28:32] += Whh_m[rows, :]
    W[2][28:32, 0:4] += Wih_m[8:12, :]
    W[3][28:32, 28:32] += Whh_m[8:12, :]

    def unit_bias(vec_j, vec_m, rows):
        b = np.zeros(S, np.float64)
        for u in range(7):
            b[st(u)] = vec_j[rows]
        b[28:32] = vec_m[rows]
        return b

    br = unit_bias(p["bih_j"], p["bih_m"], slice(0, 4)) + unit_bias(
        p["bhh_j"], p["bhh_m"], slice(0, 4))
    bz = unit_bias(p["bih_j"], p["bih_m"], slice(4, 8)) + unit_bias(
        p["bhh_j"], p["bhh_m"], slice(4, 8))
    binn = unit_bias(p["bih_j"], p["bih_m"], slice(8, 12))
    bhn = unit_bias(p["bhh_j"], p["bhh_m"], slice(8, 12))
    return W, (br, bz, binn, bhn)


def _a0_ext(p):
    """[32, 19] initial-linear map: state0 = A0e @ [x(18); 1]."""
    A = np.zeros((S, 19), np.float64)
    Wj, bj, Wm, bm = p["Wj"], p["bj"], p["Wm"], p["bm"]
    for u in range(7):
        A[4 * u:4 * u + 4, 4 + u] = Wj[:, 0]
        A[4 * u:4 * u + 4, 11 + u] = Wj[:, 1]
        A[4 * u:4 * u + 4, 18] = bj
    A[28:32, 0:4] = Wm
    A[28:32, 18] = bm
    return A


def _host_weights(inputs):
    p = {k: np.asarray(v, np.float64) for k, v in inputs.items() if k != "x"}
    W, (br, bz, binn, bhn) = _gate_blocks(p)
    A0e = _a0_ext(p)

    # wtb [128,128]: rows 32q+k (k<32) = state idx, cols 32g+m = gate out m of block g
    wtb = np.zeros((128, 128), np.float64)
    # wt1 [128,128]: iteration-1 gate weights consuming xe(19) directly
    wt1 = np.zeros((128, 128), np.float64)
    # a0t: diag blocks for S0 psum (iter-1 blend h operand)
    a0t = np.zeros((128, 128), np.float64)
    # wat: diag blocks for output linear (state -> 7 activations)
    wat = np.zeros((128, 128), np.float64)
    Wa = p["Wa"]  # [1, 4]
    for q in range(4):
        r0 = 32 * q
        for g in range(4):
            wtb[r0:r0 + 32, 32 * g:32 * g + 32] = W[g].T
            W1g = W[g] @ A0e  # [32, 19]
            wt1[r0:r0 + 19, 32 * g:32 * g + 32] = W1g.T
        a0t[r0:r0 + 19, r0:r0 + 32] = A0e.T
        for u in range(7):
            wat[r0 + 4 * u:r0 + 4 * u + 4, r0 + u] = Wa[0, :]

    def bias128(v):
        return np.tile(v, 4).astype(np.float32).reshape(128, 1)

    return {
        "wtb": wtb.astype(BF16), "wt1": wt1.astype(BF16),
        "a0t": a0t.astype(BF16), "wat": wat.astype(BF16),
        "br": bias128(br), "bz": bias128(bz),
        "binn": bias128(binn), "bhn": bias128(bhn),
    }, float(np.asarray(inputs["ba"]).reshape(-1)[0])


def _host_x(x):
    """x [B,18] fp32 -> per-core [128, NCOL] bf16 (partition 32q+k, k<19)."""
    xs = []
    for c in range(N_CORES):
        xc = np.asarray(x[c * BC:(c + 1) * BC], np.float32)
        arr = np.zeros((4, 32, NCOL), np.float32)
        arr[:, 0:18, :] = xc.reshape(4, NCOL, 18).transpose(0, 2, 1)
        arr[:, 18, :] = 1.0
        xs.append(arr.reshape(128, NCOL).astype(BF16))
    return xs


def _build_program(ncol=NCOL, nt=NT):
    nc = bacc.Bacc("TRN2", target_bir_lowering=False, debug=False,
                   num_devices=N_CORES)
    f32 = mybir.dt.float32
    bf16 = mybir.dt.bfloat16

    xd = nc.dram_tensor("x_il", [128, ncol], bf16, kind="ExternalInput").ap()
    wtbd = nc.dram_tensor("wtb", [128, 128], bf16, kind="ExternalInput").ap()
    wt1d = nc.dram_tensor("wt1", [128, 128], bf16, kind="ExternalInput").ap()
    a0td = nc.dram_tensor("a0t", [128, 128], bf16, kind="ExternalInput").ap()
    watd = nc.dram_tensor("wat", [128, 128], bf16, kind="ExternalInput").ap()
    biasd = {k: nc.dram_tensor(k, [128, 1], f32, kind="ExternalInput").ap()
             for k in ("br", "bz", "binn", "bhn")}
    yd = nc.dram_tensor("y", [28, ncol], f32, kind="ExternalOutput").ap()

    with tile.TileContext(nc) as tc, ExitStack() as ctx:
        cpool = ctx.enter_context(tc.tile_pool(name="consts", bufs=1))
        spool = ctx.enter_context(tc.tile_pool(name="state", bufs=1))
        gpool = ctx.enter_context(tc.tile_pool(name="gates", bufs=4))
        opool = ctx.enter_context(tc.tile_pool(name="outsb", bufs=3))

        xt = spool.tile([128, ncol], bf16, tag="xt")
        nc.sync.dma_start(xt[:], xd[:])
        # Per-supertile state tiles: avoids whole-tensor false dependencies
        # that would serialize the supertile pipeline.
        sts = [spool.tile([128, CT], bf16, name=f"st{t}", tag=f"st{t}")
               for t in range(nt)]

        wtb = cpool.tile([128, 128], bf16, tag="wtb")
        nc.sync.dma_start(wtb[:], wtbd[:])
        wt1 = cpool.tile([128, 128], bf16, tag="wt1")
        nc.sync.dma_start(wt1[:], wt1d[:])
        a0t = cpool.tile([128, 128], bf16, tag="a0t")
        nc.sync.dma_start(a0t[:], a0td[:])
        wat = cpool.tile([128, 128], bf16, tag="wat")
        nc.sync.dma_start(wat[:], watd[:])
        bias = {}
        for k in ("br", "bz", "binn", "bhn"):
            bias[k] = cpool.tile([128, 1], f32, tag=k, name=f"b_{k}")
            nc.sync.dma_start(bias[k][:], biasd[k][:])

        def step(it, psg, psum_s0_pool):
            """One message-passing iteration over all supertiles."""
            first = it == 0
            wt = wt1 if first else wtb
            kk = 19 if first else 32
            for t in range(nt):
                cols = slice(t * CT, (t + 1) * CT)
                G = psg.tile([128, 4 * CT], f32, tag="G")
                for g in range(4):
                    for q in range(4):
                        r0 = 32 * q
                        rhs = (xt[r0:r0 + kk, cols] if first
                               else sts[t][r0:r0 + kk, :])
                        nc.tensor.matmul(
                            G[r0:r0 + 32, g * CT:(g + 1) * CT],
                            wt[r0:r0 + kk, 32 * g:32 * g + 32],
                            rhs,
                            start=True, stop=True,
                            tile_position=(r0, r0),
                        )
                if first:
                    S0 = psum_s0_pool.tile([128, CT], f32, tag="S0")
                    for q in range(4):
                        r0 = 32 * q
                        nc.tensor.matmul(
                            S0[r0:r0 + 32, :],
                            a0t[r0:r0 + 19, r0:r0 + 32],
                            xt[r0:r0 + 19, cols],
                            start=True, stop=True,
                            tile_position=(r0, r0),
                        )
                r = gpool.tile([128, CT], bf16, tag="r")
                nc.scalar.activation(r[:], G[:, 0:CT], AF.Sigmoid,
                                     bias=bias["br"][:])
                z = gpool.tile([128, CT], bf16, tag="z")
                nc.scalar.activation(z[:], G[:, CT:2 * CT], AF.Sigmoid,
                                     bias=bias["bz"][:])
                t1 = gpool.tile([128, CT], bf16, tag="t1")
                nc.vector.scalar_tensor_tensor(
                    t1[:], G[:, 3 * CT:4 * CT], bias["bhn"][:], r[:],
                    ALU.add, ALU.mult)
                t2 = gpool.tile([128, CT], bf16, tag="t2")
                nc.vector.tensor_add(t2[:], t1[:], G[:, 2 * CT:3 * CT])
                n = gpool.tile([128, CT], bf16, tag="n")
                nc.scalar.activation(n[:], t2[:], AF.Tanh,
                                     bias=bias["binn"][:])
                d = gpool.tile([128, CT], bf16, tag="d")
                if first:
                    nc.vector.tensor_sub(d[:], S0[:], n[:])
                else:
                    nc.gpsimd.tensor_sub(d[:], sts[t][:], n[:])
                e = gpool.tile([128, CT], bf16, tag="e")
                if t % 8 < 1:
                    nc.gpsimd.tensor_mul(e[:], z[:], d[:])
                else:
                    nc.vector.tensor_mul(e[:], z[:], d[:])
                nc.vector.tensor_add(sts[t][:], n[:], e[:])

        with tc.tile_pool(name="ps1", bufs=1, space="PSUM") as psg1, \
             tc.tile_pool(name="ps0", bufs=2, space="PSUM") as ps0:
            step(0, psg1, ps0)
        with tc.tile_pool(name="psg", bufs=2, space="PSUM") as psg:
            for it in range(1, 7):
                step(it, psg, None)
        with tc.tile_pool(name="pso", bufs=2, space="PSUM") as pso:
            for t in range(nt):
                cols = slice(t * CT, (t + 1) * CT)
                O = pso.tile([128, CT], f32, tag="O")
                for q in range(4):
                    r0 = 32 * q
                    nc.tensor.matmul(
                        O[r0:r0 + 32, :],
                        wat[r0:r0 + 28, r0:r0 + 32],
                        sts[t][r0:r0 + 28, :],
                        start=True, stop=True,
                        tile_position=(r0, r0),
                    )
                osb = opool.tile([128, CT], f32, tag="osb")
                nc.scalar.copy(osb[:], O[:])
                for q in range(4):
                    nc.sync.dma_start(yd[7 * q:7 * q + 7, cols],
                                      osb[32 * q:32 * q + 7, :])

    nc.compile()
    return nc


_NC_CACHE = {}


def kernel(**inputs):
    x = np.asarray(inputs["x"])
    wd, ba = _host_weights(inputs)
    xs = _host_x(x)

    if "prog" not in _NC_CACHE:
        _NC_CACHE["prog"] = _build_program()
    nc = _NC_CACHE["prog"]

    in_maps = []
    for c in range(N_CORES):
        m = {"x_il": xs[c]}
        m.update({k: wd[k] for k in ("wtb", "wt1", "a0t", "wat",
                                     "br", "bz", "binn", "bhn")})
        in_maps.append(m)

    res = run_bass_kernel_spmd(nc, in_maps, core_ids=list(range(N_CORES)))
    _NC_CACHE["last_result"] = res
    outs = []
    for c in range(N_CORES):
        yc = np.asarray(res.results[c]["y"], np.float32)  # [28, NCOL]
        oc = yc.reshape(4, 7, NCOL).transpose(0, 2, 1).reshape(BC, 7)
        outs.append(oc)
    out = np.concatenate(outs, 0).reshape(B, 7, 1) + np.float32(ba)
    return out.astype(np.float32)


if __name__ == "__main__":
    rng = np.random.default_rng(0)
    demo = {"x": rng.standard_normal((B, 18), dtype=np.float32)}
    for k, shp in [("Wj", (H, 2)), ("bj", (H,)), ("Wm", (H, H)), ("bm", (H,)),
                   ("Wih_j", (3 * H, 2 * H)), ("Whh_j", (3 * H, H)),
                   ("bih_j", (3 * H,)), ("bhh_j", (3 * H,)),
                   ("Wih_m", (3 * H, H)), ("Whh_m", (3 * H, H)),
                   ("bih_m", (3 * H,)), ("bhh_m", (3 * H,)),
                   ("Wa", (1, H)), ("ba", (1,))]:
        demo[k] = (rng.standard_normal(shp) * 0.1).astype(np.float32)
    y = kernel(**demo)
    print(y.shape, y.dtype)
